# revision 1
# baseline (speedup 1.0000x reference)
"""Trainium2 Bass kernel for single-head attention (B=4, S=4096, C=D=512).

Sharding: 8 cores = 4 batches x 2 query-halves. Each core:
  - receives x ROLLED so its query half occupies rows 0..2047 (attention
    over keys is order-invariant, so rolling keys is exact),
  - projects K/V for the full 4096 keys (duplicated across the two cores
    of a batch pair; a pair AllGather was measured at ~112us of rendezvous
    latency on this runtime, far more than the ~30us of duplicate work),
  - projects Q for rows 0..2047 reusing the phase-A transposes,
  - computes softmax(Q K^T / sqrt(D)) V and the output projection.

On-chip layout notes:
  - x is passed bf16 (projection matmuls are bf16 anyway), halving x DMA
    traffic; it is transposed on the PE (128x128 transposes, bf16 PSUM)
    into xT[d, rows]. The transposes for rows 0..2047 persist in SBUF and
    feed the Q projection directly (no second transpose pass).
  - bk drops out (a per-query constant shift of scores cancels in
    softmax); bv folds into bo on the host (bo_eff = bo + bv @ Wo), so
    K/V projections need no bias work on-chip. bo_eff enters via a K=1
    ones-row matmul into the output-projection accumulation group.
  - Scores are computed transposed (scoreT[s, q]) so exp(scoreT) feeds
    the attnT matmul directly with no per-tile transposes.
  - Row sums l[q] are accumulated on the DVE (l_sb += pT per key tile),
    freeing ~27us of PE time vs a ones-matmul per key tile; 1/l is
    applied per query row via the ScalarE copy's per-partition scale AP.
  - PSUM->SBUF copies for kT/vv/qT/attnT/ot run on the Scalar engine
    (otherwise idle) so the DVE never becomes the bottleneck.
  - The s-loop is software-pipelined: score matmuls for key-tile st+1/st+2
    are issued before the exp(st)-consuming matmuls so the in-order PE
    never waits on the ScalarE.
"""

import sys

for _p in ("/opt/trn_rl_repo", "/root/.axon_site/_ro/trn_rl_repo"):
    if _p not in sys.path:
        sys.path.append(_p)

import numpy as np
import ml_dtypes
import concourse.bacc as bacc
import concourse.mybir as mybir
import concourse.tile as tile
from concourse.bass_utils import run_bass_kernel_spmd
from concourse.masks import make_identity

F32 = mybir.dt.float32
BF16 = mybir.dt.bfloat16

MM_DT = BF16

B, S, C, D = 4, 4096, 512, 512
Q = S // 2          # queries per core
N_CORES = 8
SCALE = float(D) ** -0.5
QB = 512            # query block (psum bank width in fp32)
N_QB = Q // QB      # 4 query blocks per core
N_ST = S // 128     # 32 key tiles
N_DC = C // 128     # 4 contraction chunks
N_RG = S // 512     # 8 row groups


def _build_program():
    nc = bacc.Bacc(None, target_bir_lowering=False, debug=False)

    x = nc.dram_tensor("x", [S, C], BF16, kind="ExternalInput")
    w_dram = {
        name: nc.dram_tensor(name, [C, D], F32, kind="ExternalInput")
        for name in ("Wq", "Wk", "Wv", "Wo")
    }
    bq_dram = nc.dram_tensor("bq", [D], F32, kind="ExternalInput")
    bo_dram = nc.dram_tensor("bo", [D], F32, kind="ExternalInput")
    out = nc.dram_tensor("out", [Q, D], F32, kind="ExternalOutput")

    ActFn = mybir.ActivationFunctionType

    with tile.TileContext(nc) as tc:
        persist = tc.alloc_tile_pool(name="persist", bufs=1)
        const = tc.alloc_tile_pool(name="const", bufs=1)
        wkv_pool = tc.alloc_tile_pool(name="wkv", bufs=1)
        wraw_pool = tc.alloc_tile_pool(name="wraw", bufs=2)

        # ---- constants needed by the first transposes ----
        identity = const.tile([128, 128], BF16, tag="identity")
        make_identity(nc, identity[:])
        ones_f32 = const.tile([128, 128], F32, tag="ones_f32")
        nc.vector.memset(ones_f32[:], 1.0)

        wts = {}

        def emit_weight(name, engine):
            pool = wkv_pool if name in ("Wk", "Wv") else persist
            wt = pool.tile([128, N_DC, D], MM_DT, tag=f"w_{name}", name=f"w_{name}")
            for dc in range(N_DC):
                raw = wraw_pool.tile([128, D], F32, tag="raw", name=f"raw_{name}{dc}")
                engine.dma_start(raw[:], w_dram[name][dc * 128 : (dc + 1) * 128, :])
                nc.vector.tensor_copy(wt[:, dc, :], raw[:])
            wts[name] = wt

        # ---- persistent activations ----
        kT = persist.tile([128, N_DC, S], MM_DT, tag="kT")     # kT[p, dc, s] = K[s, dc*128+p]
        vv = persist.tile([128, N_ST, D], MM_DT, tag="v")      # vv[p, i, e] = V[i*128+p, e]
        # transposed x for the query half (rows 0..2047), reused for Q proj
        xqT = [persist.tile([128, N_DC, 512], MM_DT, tag=f"xqT{i}", name=f"xqT{i}")
               for i in range(N_QB)]

        # ================= phase A: K/V projections =================
        xload = tc.alloc_tile_pool(name="xload", bufs=4)
        xTgp = tc.alloc_tile_pool(name="xTg", bufs=3)
        ps_tr = tc.alloc_tile_pool(name="ps_tr", bufs=3, space="PSUM")
        ps_proj = tc.alloc_tile_pool(name="ps_proj", bufs=4, space="PSUM")

        def emit_xT(rg):
            # bf16 loads + PE transposes of rows [rg*512, (rg+1)*512);
            # query-half rows persist in xqT for reuse by the Q projection.
            # One batched DMA per row group (rg0 split fine for fast start),
            # alternating between the two HWDGE queues.
            if rg < N_QB:
                xTg = xqT[rg]
            else:
                xTg = xTgp.tile([128, N_DC, 512], MM_DT, tag="xTg", name=f"xTg{rg}")
            xt = xload.tile([128, 4, 512], BF16, tag="xt", name=f"xt{rg}")
            if rg == 0:
                for rt in range(4):
                    eng = nc.sync if rt % 2 == 0 else nc.scalar
                    eng.dma_start(xt[:, rt, :],
                                  x[(rg * 4 + rt) * 128 : (rg * 4 + rt + 1) * 128, :])
            else:
                eng = nc.sync if rg % 2 == 0 else nc.scalar
                eng.dma_start(
                    xt[:],
                    x[rg * 512 : (rg + 1) * 512, :].rearrange("(j p) c -> p j c", j=4))
            for rt in range(4):
                pt = ps_tr.tile([128, 512], BF16, tag="pt", name=f"pt{rg}_{rt}")
                for dc in range(N_DC):
                    nc.tensor.transpose(pt[:, dc * 128 : (dc + 1) * 128],
                                        xt[:, rt, dc * 128 : (dc + 1) * 128], identity[:])
                nc.vector.tensor_copy(
                    xTg[:, :, rt * 128 : rt * 128 + 128],
                    pt[:].rearrange("p (a b) -> p a b", a=N_DC),
                )
            return xTg

        # x row-group 0 split across both HWDGE queues; Wk right behind on
        # scalar (needed by rg0's K-proj), Wv after it (rg0's V-proj).
        pipe = [emit_xT(0)]
        emit_weight("Wk", nc.scalar)
        pipe.append(emit_xT(1))

        # pre-load the ACT exp table during phase-A DMA waits
        warm = const.tile([1, 1], F32, tag="warm")
        nc.scalar.activation(warm[:], ones_f32[0:1, 0:1], ActFn.Exp, scale=1.0)

        # remaining weights on the fast HWDGE queues (SWDGE measured ~43GB/s,
        # far too slow for the phase-A weight convoy); tiny 4B-element bias
        # DMAs go on the vector queue so they can't block anything.
        emit_weight("Wv", nc.scalar)
        ones_r = const.tile([128, 128], MM_DT, tag="ones_r")
        nc.vector.tensor_copy(ones_r[:], ones_f32[:])
        bqT = const.tile([128, N_DC], F32, tag="bqT")
        bo_raw = const.tile([1, D], F32, tag="bo_raw")
        bo_bf = const.tile([1, D], MM_DT, tag="bo_bf")

        pipe.append(emit_xT(2))

        for rg in range(N_RG):              # 8 row groups of 512 rows
            xTg = pipe.pop(0)
            if rg + 3 < N_RG:
                pipe.append(emit_xT(rg + 3))
            # kT for these 512 rows (bk cancels in softmax: pure matmul)
            for g in range(N_DC):
                pk = ps_proj.tile([128, 512], F32, tag="pk")
                for dc in range(N_DC):
                    nc.tensor.matmul(pk[:], wts["Wk"][:, dc, g * 128 : (g + 1) * 128],
                                     xTg[:, dc, :], start=(dc == 0), stop=(dc == N_DC - 1))
                nc.scalar.activation(kT[:, g, rg * 512 : (rg + 1) * 512], pk[:],
                                     ActFn.Copy)
            # V for these 512 rows (bv folded into bo_eff on the host)
            for rt in range(4):
                pv = ps_proj.tile([128, 512], F32, tag="pk")
                for dc in range(N_DC):
                    nc.tensor.matmul(pv[:], xTg[:, dc, rt * 128 : (rt + 1) * 128],
                                     wts["Wv"][:, dc, :], start=(dc == 0), stop=(dc == N_DC - 1))
                nc.scalar.activation(vv[:, rg * 4 + rt, :], pv[:], ActFn.Copy)
            if rg == 3:
                # deferred past the early HBM-saturated window (x row-groups +
                # Wk/Wv demand ~290GB/s of the 358 limit in the first 25us);
                # still lands well before phase B needs them
                emit_weight("Wq", nc.sync)
                emit_weight("Wo", nc.sync)
            if rg == 5:
                # tiny 4B-element bias DMAs, needed only at phase-B start
                for g in range(N_DC):
                    nc.gpsimd.dma_start(bqT[:, g : g + 1],
                                        bq_dram[g * 128 : (g + 1) * 128].unsqueeze(1))
                nc.gpsimd.dma_start(bo_raw[0:1, :], bo_dram[:].unsqueeze(0))
                nc.vector.tensor_copy(bo_bf[:], bo_raw[:])

        ps_proj.release(); ps_tr.release()
        xTgp.release(); xload.release()
        wraw_pool.release(); wkv_pool.release()

        # ================= phase B: attention =================
        with tc.tile_pool(name="qT", bufs=2) as qTp, \
             tc.tile_pool(name="pT", bufs=8) as pTp, \
             tc.tile_pool(name="rl", bufs=2) as rlp, \
             tc.tile_pool(name="attnT", bufs=2) as attnTp, \
             tc.tile_pool(name="osb", bufs=4) as osbp, \
             tc.tile_pool(name="ps_at", bufs=4, space="PSUM") as ps_atp, \
             tc.tile_pool(name="ps_s", bufs=3, space="PSUM") as ps_sp, \
             tc.tile_pool(name="ps_l", bufs=1, space="PSUM") as ps_lp:

            def emit_qproj(qb):
                # Q projection for one 512-query block, from the saved
                # phase-A transpose of rows [qb*512, (qb+1)*512).
                xqTg = xqT[qb]
                qT = qTp.tile([128, N_DC, 512], MM_DT, tag="qT", name=f"qT{qb}")
                for g in range(N_DC):
                    pq = ps_sp.tile([128, 512], F32, tag="ss", name=f"pq{qb}_{g}")
                    for dc in range(N_DC):
                        nc.tensor.matmul(pq[:], wts["Wq"][:, dc, g * 128 : (g + 1) * 128],
                                         xqTg[:, dc, :], start=(dc == 0), stop=(dc == N_DC - 1))
                    nc.scalar.activation(qT[:, g, :], pq[:], ActFn.Identity,
                                         bias=bqT[:, g : g + 1])
                return qT

            def emit_score(qb, st, qT):
                ss = ps_sp.tile([128, 512], F32, tag="ss", name=f"ss{qb}_{st}")
                for dc in range(N_DC):
                    nc.tensor.matmul(ss[:], kT[:, dc, st * 128 : (st + 1) * 128],
                                     qT[:, dc, :], start=(dc == 0), stop=(dc == N_DC - 1))
                return ss

            qT_cur = emit_qproj(0)
            for qb in range(N_QB):
                qT = qT_cur
                l_sb = rlp.tile([128, 512], F32, tag="l_sb", name=f"lsb{qb}")
                at_ps = [ps_atp.tile([128, 512], F32, tag="at", name=f"at{qb}_{et}")
                         for et in range(4)]
                ss_q = [emit_score(qb, 0, qT), emit_score(qb, 1, qT)]
                for st in range(N_ST):
                    if st + 2 < N_ST:
                        ss_q.append(emit_score(qb, st + 2, qT))
                    ss = ss_q.pop(0)
                    pT = pTp.tile([128, 512], MM_DT, tag="pT", name=f"pT{qb}_{st}")
                    nc.scalar.activation(pT[:], ss[:], ActFn.Exp, scale=SCALE)
                    for et in range(4):
                        nc.tensor.matmul(at_ps[et][:], vv[:, st, et * 128 : (et + 1) * 128],
                                         pT[:], start=(st == 0), stop=(st == N_ST - 1))
                    # row-sum accumulation on the DVE (off the PE)
                    if st == 0:
                        nc.vector.tensor_copy(l_sb[:], pT[:])
                    else:
                        nc.vector.tensor_add(l_sb[:], l_sb[:], pT[:])

                if qb + 1 < N_QB:
                    qT_cur = emit_qproj(qb + 1)

                # --- epilogue: 1/l arranged with queries on partitions
                # ([128,4] via tiny transposing matmuls -> fast reciprocal),
                # applied per query row by the ScalarE copy's scale AP ---
                lbf = rlp.tile([128, 512], MM_DT, tag="lbf", name=f"lbf{qb}")
                nc.vector.tensor_copy(lbf[:], l_sb[:])
                l_ps = ps_lp.tile([128, 512], F32, tag="l", name=f"l{qb}")
                nc.tensor.matmul(l_ps[:], ones_r[:], lbf[:])
                l_row = rlp.tile([1, 512], F32, tag="l_row", name=f"lrow{qb}")
                nc.vector.tensor_copy(l_row[:], l_ps[0:1, :])
                lt_ps = ps_lp.tile([128, 4], F32, tag="l", name=f"lt{qb}")
                for rt in range(4):
                    nc.tensor.matmul(lt_ps[:, rt : rt + 1],
                                     l_row[0:1, rt * 128 : (rt + 1) * 128],
                                     ones_f32[0:1, 0:1])
                rlT = rlp.tile([128, 4], F32, tag="rlT", name=f"rlT{qb}")
                nc.vector.reciprocal(rlT[:], lt_ps[:])

                attnT = attnTp.tile([128, 4, 512], MM_DT, tag="attnT", name=f"attnT{qb}")
                for et in range(4):
                    nc.vector.tensor_copy(attnT[:, et, :], at_ps[et][:])
                # all four po banks from the at pool (its tiles free as the
                # attnT copies land), keeping the ss pool clear for the next
                # query block's score pipeline
                po = [ps_atp.tile([128, 512], F32, tag="at", name=f"po{qb}_{rt}")
                      for rt in range(4)]
                # bias row opens each accumulation group (it has no deps);
                # ec-outer after, so po[rt] completes at its ec=3 matmul and
                # the final scaled copies + output DMAs pipeline per rt
                for rt in range(4):
                    nc.tensor.matmul(po[rt][:], ones_r[0:1, :], bo_bf[0:1, :],
                                     start=True, stop=False)
                for ec in range(4):
                    for rt in range(4):
                        nc.tensor.matmul(po[rt][:], attnT[:, ec, rt * 128 : (rt + 1) * 128],
                                         wts["Wo"][:, ec, :], start=False, stop=(ec == 3))
                        if ec == 3:
                            ot = osbp.tile([128, D], F32, tag="ot", name=f"ot{qb}_{rt}")
                            # ot = po * (1/l[row]) on the DVE (ScalarE is busy
                            # with the next block's qT copies + exps here)
                            nc.vector.tensor_scalar_mul(ot[:], po[rt][:], rlT[:, rt : rt + 1])
                            nc.sync.dma_start(out[(qb * 4 + rt) * 128 : (qb * 4 + rt + 1) * 128, :], ot[:])

        const.release()
        persist.release()

    nc.compile()
    return nc


_NC_CACHE = None


def _get_nc():
    global _NC_CACHE
    if _NC_CACHE is None:
        _NC_CACHE = _build_program()
    return _NC_CACHE


def kernel(**inputs):
    x = np.asarray(inputs["x"], dtype=np.float32)
    xt = np.ascontiguousarray(x.reshape(B, S, C)).astype(ml_dtypes.bfloat16)
    ws = {k: np.ascontiguousarray(np.asarray(inputs[k], dtype=np.float32))
          for k in ("Wq", "Wk", "Wv", "Wo")}
    bq = np.ascontiguousarray(np.asarray(inputs["bq"], dtype=np.float32))
    # bv shifts every attention output row by a constant, so it folds into
    # the output bias: out = attn@Wo + (bo + bv@Wo).  bk cancels in softmax.
    bo_eff = np.ascontiguousarray(
        np.asarray(inputs["bo"], dtype=np.float32)
        + np.asarray(inputs["bv"], dtype=np.float32) @ ws["Wo"])

    in_maps = []
    for c in range(N_CORES):
        b, h = divmod(c, 2)
        xb = xt[b]
        if h:
            # roll keys so this core's query half occupies rows 0..2047;
            # attention over keys is order-invariant so this is exact.
            xb = np.concatenate([xb[Q:], xb[:Q]], axis=0)
        m = {"x": np.ascontiguousarray(xb), "bq": bq, "bo": bo_eff}
        m.update(ws)
        in_maps.append(m)

    nc = _get_nc()
    try:
        res = run_bass_kernel_spmd(nc, in_maps, core_ids=list(range(N_CORES)))
    except Exception:
        # transient NRT/device hiccups recover on retry
        import time
        time.sleep(15)
        res = run_bass_kernel_spmd(nc, in_maps, core_ids=list(range(N_CORES)))

    out = np.empty((B, S, D), dtype=np.float32)
    for c in range(N_CORES):
        b, h = divmod(c, 2)
        out[b, h * Q : (h + 1) * Q] = res.results[c]["out"]
    return out.reshape(B, 64, 64, D)



# revision 3
# speedup vs baseline: 1.0723x; 1.0723x over previous
"""Trainium2 Bass kernel for single-head attention (B=4, S=4096, C=D=512).

Sharding: 8 cores = 4 batches x 2 query-halves. Each core:
  - receives xT ([C, S], host-pre-transposed, bf16) ROLLED so its query
    half occupies columns 0..2047 (attention over keys is order-invariant,
    so rolling keys is exact),
  - projects K/V for the full 4096 keys (duplicated across the two cores
    of a batch pair; a pair AllGather was measured at ~112us of rendezvous
    latency on this runtime, far more than the ~30us of duplicate work),
  - projects Q for columns 0..2047 straight from the resident xT,
  - computes softmax(Q K^T / sqrt(D)) V and the output projection.

On-chip layout notes:
  - x is transposed on the HOST and passed bf16 as xT[c, s], so the
    projection matmuls consume it directly: no PE transposes, no
    PSUM->SBUF transpose copies, and the DMA'd tile is itself the
    persistent operand (32KB/partition).
  - Weights are cast to bf16 on the host and DMA'd straight into their
    persistent SBUF tiles (no on-chip f32->bf16 casts).
  - bk drops out (a per-query constant shift of scores cancels in
    softmax); bv+bo fold into a host-side bo_eff added after gather
    (biases are spec'd zero, and the host add is exact regardless), so
    the PE spends zero matmuls on bias work. bq stays on-chip, fused
    into the qT copy's bias port (free).
  - rg0's xT/Wk DMAs are split per 128-row chunk across both HWDGE
    queues and rg0's K-projection is issued dc-outer, so the first
    matmul needs only 256KB of DMA traffic.
  - Scores are computed transposed (scoreT[s, q]) so exp(scoreT) feeds
    the attnT matmul directly with no per-tile transposes.
  - Row sums l[q] are accumulated on the DVE (l_sb += pT per key tile),
    freeing ~27us of PE time vs a ones-matmul per key tile; 1/l is
    applied per query row via a per-partition scale AP.
  - PSUM->SBUF copies for kT/vv/qT run on the Scalar engine (otherwise
    idle) so the DVE never becomes the bottleneck.
  - The s-loop is software-pipelined: score matmuls for key-tile st+1/st+2
    are issued before the exp(st)-consuming matmuls so the in-order PE
    never waits on the ScalarE.
  - The output projection is rt-outer so po[rt] completes early and its
    scaled copy + output DMA (alternating queues) overlap the remaining
    matmuls, shortening the kernel tail.
"""

import sys

for _p in ("/opt/trn_rl_repo", "/root/.axon_site/_ro/trn_rl_repo"):
    if _p not in sys.path:
        sys.path.append(_p)

import numpy as np
import ml_dtypes
import concourse.bacc as bacc
import concourse.mybir as mybir
import concourse.tile as tile
from concourse.bass_utils import run_bass_kernel_spmd

F32 = mybir.dt.float32
BF16 = mybir.dt.bfloat16

MM_DT = BF16

B, S, C, D = 4, 4096, 512, 512
Q = S // 2          # queries per core
N_CORES = 8
SCALE = float(D) ** -0.5
QB = 512            # query block (psum bank width in fp32)
N_QB = Q // QB      # 4 query blocks per core
N_ST = S // 128     # 32 key tiles
N_DC = C // 128     # 4 contraction chunks
N_RG = S // 512     # 8 row groups


def _build_program():
    nc = bacc.Bacc(None, target_bir_lowering=False, debug=False)

    x = nc.dram_tensor("x", [C, S], BF16, kind="ExternalInput")   # host-transposed
    w_dram = {
        name: nc.dram_tensor(name, [C, D], BF16, kind="ExternalInput")
        for name in ("Wq", "Wk", "Wv", "Wo")
    }
    bq_dram = nc.dram_tensor("bq", [D], F32, kind="ExternalInput")
    out = nc.dram_tensor("out", [Q, D], F32, kind="ExternalOutput")

    ActFn = mybir.ActivationFunctionType

    with tile.TileContext(nc) as tc:
        persist = tc.alloc_tile_pool(name="persist", bufs=1)
        const = tc.alloc_tile_pool(name="const", bufs=1)
        wkv_pool = tc.alloc_tile_pool(name="wkv", bufs=1)

        ones_f32 = const.tile([128, 128], F32, tag="ones_f32")
        nc.vector.memset(ones_f32[:], 1.0)

        wts = {}

        def emit_weight(name, engine):
            # bf16 weights DMA'd straight into the persistent tile, one
            # DMA per 128-row contraction chunk (fine-grained deps).
            pool = wkv_pool if name in ("Wk", "Wv") else persist
            wt = pool.tile([128, N_DC, D], MM_DT, tag=f"w_{name}", name=f"w_{name}")
            for dc in range(N_DC):
                engine.dma_start(wt[:, dc, :], w_dram[name][dc * 128 : (dc + 1) * 128, :])
            wts[name] = wt

        # ---- persistent activations ----
        kT = persist.tile([128, N_DC, S], MM_DT, tag="kT")     # kT[p, dc, s] = K[s, dc*128+p]
        vv = persist.tile([128, N_ST, D], MM_DT, tag="v")      # vv[p, i, e] = V[i*128+p, e]
        xT = persist.tile([128, N_DC, S], MM_DT, tag="xT")     # xT[p, dc, s] = x[s, dc*128+p]

        # ================= phase A: K/V projections =================
        ps_proj = tc.alloc_tile_pool(name="ps_proj", bufs=4, space="PSUM")

        # rg0 x chunks on sync, Wk chunks on scalar: the dc0 pair (256KB)
        # unblocks the first K matmul.
        for dc in range(N_DC):
            nc.sync.dma_start(xT[:, dc, 0:512], x[dc * 128 : (dc + 1) * 128, 0:512])
        emit_weight("Wk", nc.scalar)

        # pre-load the ACT exp table during phase-A DMA waits
        warm = const.tile([1, 1], F32, tag="warm")
        nc.scalar.activation(warm[:], ones_f32[0:1, 0:1], ActFn.Exp, scale=1.0)

        emit_weight("Wv", nc.scalar)
        ones_r = const.tile([128, 128], MM_DT, tag="ones_r")
        nc.vector.tensor_copy(ones_r[:], ones_f32[:])
        bqT = const.tile([128, N_DC], F32, tag="bqT")

        def emit_xdma(rg, eng):
            eng.dma_start(
                xT[:, :, rg * 512 : (rg + 1) * 512],
                x[:, rg * 512 : (rg + 1) * 512].rearrange("(dc p) s -> p dc s", dc=N_DC))

        # remaining x row groups: rg1/3/5/7 on sync behind rg0, rg2/4/6 on
        # scalar behind Wk/Wv; Wq/Wo enter the sync queue at rg==3 (needed
        # only at phase-B start, after the early HBM-saturated window).
        emit_xdma(1, nc.sync)

        for rg in range(N_RG):              # 8 row groups of 512 rows
            # kT for these 512 rows (bk cancels in softmax: pure matmul).
            # rg0 runs dc-outer so the first matmuls need only the dc0 DMAs.
            pk = [ps_proj.tile([128, 512], F32, tag="pk", name=f"pk{rg}_{g}")
                  for g in range(N_DC)]
            if rg == 0:
                for dc in range(N_DC):
                    for g in range(N_DC):
                        nc.tensor.matmul(pk[g][:], wts["Wk"][:, dc, g * 128 : (g + 1) * 128],
                                         xT[:, dc, rg * 512 : (rg + 1) * 512],
                                         start=(dc == 0), stop=(dc == N_DC - 1))
            else:
                for g in range(N_DC):
                    for dc in range(N_DC):
                        nc.tensor.matmul(pk[g][:], wts["Wk"][:, dc, g * 128 : (g + 1) * 128],
                                         xT[:, dc, rg * 512 : (rg + 1) * 512],
                                         start=(dc == 0), stop=(dc == N_DC - 1))
            for g in range(N_DC):
                nc.scalar.activation(kT[:, g, rg * 512 : (rg + 1) * 512], pk[g][:],
                                     ActFn.Copy)
            # V for these 512 rows (bv folded into bo_eff on the host)
            for rt in range(4):
                pv = ps_proj.tile([128, 512], F32, tag="pk")
                for dc in range(N_DC):
                    nc.tensor.matmul(pv[:], xT[:, dc, rg * 512 + rt * 128 : rg * 512 + (rt + 1) * 128],
                                     wts["Wv"][:, dc, :], start=(dc == 0), stop=(dc == N_DC - 1))
                nc.scalar.activation(vv[:, rg * 4 + rt, :], pv[:], ActFn.Copy)
            # queue upcoming x row groups / weights while rg's matmuls run
            if rg == 0:
                emit_xdma(2, nc.scalar)
                emit_xdma(3, nc.sync)
            elif rg == 1:
                emit_xdma(4, nc.scalar)
                emit_xdma(5, nc.sync)
            elif rg == 2:
                emit_weight("Wq", nc.sync)
                emit_weight("Wo", nc.sync)
                emit_xdma(6, nc.scalar)
                emit_xdma(7, nc.sync)
            elif rg == 5:
                # tiny 4B-element bias DMAs, needed only at phase-B start
                for g in range(N_DC):
                    nc.gpsimd.dma_start(bqT[:, g : g + 1],
                                        bq_dram[g * 128 : (g + 1) * 128].unsqueeze(1))

        ps_proj.release()
        wkv_pool.release()

        # ================= phase B: attention =================
        with tc.tile_pool(name="qT", bufs=2) as qTp, \
             tc.tile_pool(name="pT", bufs=8) as pTp, \
             tc.tile_pool(name="rl", bufs=2) as rlp, \
             tc.tile_pool(name="attnT", bufs=2) as attnTp, \
             tc.tile_pool(name="osb", bufs=4) as osbp, \
             tc.tile_pool(name="ps_at", bufs=4, space="PSUM") as ps_atp, \
             tc.tile_pool(name="ps_s", bufs=3, space="PSUM") as ps_sp, \
             tc.tile_pool(name="ps_l", bufs=1, space="PSUM") as ps_lp:

            def emit_qproj(qb):
                # Q projection for one 512-query block, straight from the
                # resident xT columns [qb*512, (qb+1)*512).
                qT = qTp.tile([128, N_DC, 512], MM_DT, tag="qT", name=f"qT{qb}")
                for g in range(N_DC):
                    pq = ps_sp.tile([128, 512], F32, tag="ss", name=f"pq{qb}_{g}")
                    for dc in range(N_DC):
                        nc.tensor.matmul(pq[:], wts["Wq"][:, dc, g * 128 : (g + 1) * 128],
                                         xT[:, dc, qb * 512 : (qb + 1) * 512],
                                         start=(dc == 0), stop=(dc == N_DC - 1))
                    nc.scalar.activation(qT[:, g, :], pq[:], ActFn.Identity,
                                         bias=bqT[:, g : g + 1])
                return qT

            def emit_score(qb, st, qT):
                ss = ps_sp.tile([128, 512], F32, tag="ss", name=f"ss{qb}_{st}")
                for dc in range(N_DC):
                    nc.tensor.matmul(ss[:], kT[:, dc, st * 128 : (st + 1) * 128],
                                     qT[:, dc, :], start=(dc == 0), stop=(dc == N_DC - 1))
                return ss

            qT_cur = emit_qproj(0)
            for qb in range(N_QB):
                qT = qT_cur
                l_sb = rlp.tile([128, 512], F32, tag="l_sb", name=f"lsb{qb}")
                at_ps = [ps_atp.tile([128, 512], F32, tag="at", name=f"at{qb}_{et}")
                         for et in range(4)]
                ss_q = [emit_score(qb, 0, qT), emit_score(qb, 1, qT)]
                for st in range(N_ST):
                    if st + 2 < N_ST:
                        ss_q.append(emit_score(qb, st + 2, qT))
                    ss = ss_q.pop(0)
                    pT = pTp.tile([128, 512], MM_DT, tag="pT", name=f"pT{qb}_{st}")
                    nc.scalar.activation(pT[:], ss[:], ActFn.Exp, scale=SCALE)
                    for et in range(4):
                        nc.tensor.matmul(at_ps[et][:], vv[:, st, et * 128 : (et + 1) * 128],
                                         pT[:], start=(st == 0), stop=(st == N_ST - 1))
                    # row-sum accumulation on the DVE (off the PE)
                    if st == 0:
                        nc.vector.tensor_copy(l_sb[:], pT[:])
                    else:
                        nc.vector.tensor_add(l_sb[:], l_sb[:], pT[:])

                if qb + 1 < N_QB:
                    qT_cur = emit_qproj(qb + 1)

                # --- epilogue: 1/l arranged with queries on partitions
                # ([128,4] via tiny transposing matmuls -> fast reciprocal),
                # applied per query row by a per-partition scale AP ---
                lbf = rlp.tile([128, 512], MM_DT, tag="lbf", name=f"lbf{qb}")
                nc.vector.tensor_copy(lbf[:], l_sb[:])
                l_ps = ps_lp.tile([128, 512], F32, tag="l", name=f"l{qb}")
                nc.tensor.matmul(l_ps[:], ones_r[:], lbf[:])
                l_row = rlp.tile([1, 512], F32, tag="l_row", name=f"lrow{qb}")
                nc.vector.tensor_copy(l_row[:], l_ps[0:1, :])
                lt_ps = ps_lp.tile([128, 4], F32, tag="l", name=f"lt{qb}")
                for rt in range(4):
                    nc.tensor.matmul(lt_ps[:, rt : rt + 1],
                                     l_row[0:1, rt * 128 : (rt + 1) * 128],
                                     ones_f32[0:1, 0:1])
                rlT = rlp.tile([128, 4], F32, tag="rlT", name=f"rlT{qb}")
                nc.vector.reciprocal(rlT[:], lt_ps[:])

                attnT = attnTp.tile([128, 4, 512], MM_DT, tag="attnT", name=f"attnT{qb}")
                for et in range(4):
                    nc.vector.tensor_copy(attnT[:, et, :], at_ps[et][:])
                # rt-outer: po[rt] completes after its own 4 matmuls, so the
                # scaled copy + output DMA for rt=0 overlap rt=1..3's matmuls
                # (shortens the kernel tail after the last qb)
                for rt in range(4):
                    po = ps_atp.tile([128, 512], F32, tag="at", name=f"po{qb}_{rt}")
                    for ec in range(4):
                        nc.tensor.matmul(po[:], attnT[:, ec, rt * 128 : (rt + 1) * 128],
                                         wts["Wo"][:, ec, :], start=(ec == 0), stop=(ec == 3))
                    ot = osbp.tile([128, D], F32, tag="ot", name=f"ot{qb}_{rt}")
                    # ot = po * (1/l[row]) on the DVE (ScalarE is busy with
                    # the next block's qT copies + exps here)
                    nc.vector.tensor_scalar_mul(ot[:], po[:], rlT[:, rt : rt + 1])
                    eng = nc.sync if rt % 2 == 0 else nc.scalar
                    eng.dma_start(out[(qb * 4 + rt) * 128 : (qb * 4 + rt + 1) * 128, :], ot[:])

        const.release()
        persist.release()

    nc.compile()
    return nc


_NC_CACHE = None


def _get_nc():
    global _NC_CACHE
    if _NC_CACHE is None:
        _NC_CACHE = _build_program()
    return _NC_CACHE


def kernel(**inputs):
    x = np.asarray(inputs["x"], dtype=np.float32)
    # host-side transpose to xT[c, s] per batch, cast bf16
    xt = np.ascontiguousarray(
        x.reshape(B, S, C).transpose(0, 2, 1)).astype(ml_dtypes.bfloat16)
    ws = {k: np.asarray(inputs[k], dtype=np.float32).astype(ml_dtypes.bfloat16)
          for k in ("Wq", "Wk", "Wv", "Wo")}
    bq = np.ascontiguousarray(np.asarray(inputs["bq"], dtype=np.float32))
    # bv shifts every attention output row by a constant, so it folds into
    # the output bias: out = attn@Wo + (bo + bv@Wo), added on the host
    # after gather.  bk cancels in softmax.
    bo_eff = (np.asarray(inputs["bo"], dtype=np.float32)
              + np.asarray(inputs["bv"], dtype=np.float32)
              @ np.asarray(inputs["Wo"], dtype=np.float32))

    in_maps = []
    for c in range(N_CORES):
        b, h = divmod(c, 2)
        xb = xt[b]
        if h:
            # roll keys so this core's query half occupies columns 0..2047;
            # attention over keys is order-invariant so this is exact.
            xb = np.concatenate([xb[:, Q:], xb[:, :Q]], axis=1)
        m = {"x": np.ascontiguousarray(xb), "bq": bq}
        m.update(ws)
        in_maps.append(m)

    nc = _get_nc()
    try:
        res = run_bass_kernel_spmd(nc, in_maps, core_ids=list(range(N_CORES)))
    except Exception:
        # transient NRT/device hiccups recover on retry
        import time
        time.sleep(15)
        res = run_bass_kernel_spmd(nc, in_maps, core_ids=list(range(N_CORES)))

    out = np.empty((B, S, D), dtype=np.float32)
    for c in range(N_CORES):
        b, h = divmod(c, 2)
        out[b, h * Q : (h + 1) * Q] = res.results[c]["out"]
    if np.any(bo_eff):
        out += bo_eff
    return out.reshape(B, 64, 64, D)
